# revision 8
# baseline (speedup 1.0000x reference)
"""Local (sliding-window, causal) self-attention for Trainium2, SPMD over 8 NeuronCores.

Problem: B=4, L=4096, HIDDEN=512, 8 heads x 64 dim, causal window 32.

Sharding: data-parallel over (batch, sequence-half) -> 8 shards. Local attention
only needs a (window-1)=31-token left halo of K/V context, so each core is fully
independent (no collectives).

Device-side layout strategy (everything contracts over the SBUF partition dim):
  - Host pre-transposes activations/weights, so the QKV projection produces
    Q^T/K^T feature-major directly and V token-major, with no device transposes.
  - fp32r matmuls (full PE rate at N>=256, ~tf32 precision) for the three
    projections; bf16 for the windowed attention matmuls.
  - Scores for a 128-query block against its 160-wide key span; softmax without
    max-subtraction (scores are O(1) by construction); exp -> P; P^T via PE
    transpose; band+validity mask applied multiplicatively during the P^T
    eviction; softmax denominator = ones-column augmented into V so it falls out
    of the PV matmul per-partition; ctx normalized token-major, transposed on PE
    to feature-major for the fp32r output projection; result DMA'd out
    feature-major and un-transposed on the host.
"""

import numpy as np

B, L, HID = 4, 4096, 512
H, D, W, ATTN = 8, 64, 32, 512
NCORE = 8
NQ = L // 2                 # queries per core
HALO = W - 1                # 31
KV = HALO + NQ + 1          # 2080 kv positions incl. 1 right pad
KVP = 2176                  # hidT width, padded to 17*128
NT = KVP // 128             # 17 V token chunks
NBLK = NQ // 128            # 16 query blocks
SPAN = 160                  # key span per query block
PB = 4                      # blocks per attention pass
NPASS = NBLK // PB          # 4 passes per head
SQS = float(D) ** -0.25     # sqrt of attention scale, folded into Wq,Wk,bq,bk

A1_TILES = [(0, 512), (512, 512), (1024, 512), (1536, 512), (2048, 128)]

_compiled = None


def _build(iters=1):
    from contextlib import ExitStack

    import concourse.bass as bass
    import concourse.mybir as mybir
    import concourse.tile as tile
    from concourse import bacc
    from concourse.masks import make_identity

    f32 = mybir.dt.float32
    f32r = mybir.dt.float32r
    bf16 = mybir.dt.bfloat16
    AF = mybir.ActivationFunctionType

    nc = bacc.Bacc("TRN2", target_bir_lowering=False, debug=False,
                   num_devices=NCORE)

    hidT_d = nc.dram_tensor("hidT", [4, 128, KVP], f32r, kind="ExternalInput").ap()
    wqkT_d = nc.dram_tensor("wqkT", [4, 128, 1536], f32r, kind="ExternalInput").ap()
    woT_d = nc.dram_tensor("woT", [4, 128, 512], f32r, kind="ExternalInput").ap()
    qkbT_d = nc.dram_tensor("qkbT", [128, 8], f32, kind="ExternalInput").ap()
    vbb_d = nc.dram_tensor("vbb", [128, 512], f32, kind="ExternalInput").ap()
    mC_d = nc.dram_tensor("maskC", [128, NBLK, 256], bf16, kind="ExternalInput").ap()
    outT_d = nc.dram_tensor("outT", [4, 128, NQ], f32, kind="ExternalOutput").ap()

    def r(ap):
        return ap.bitcast(f32r)

    with tile.TileContext(nc) as tc:
        import contextlib
        loop_cm = tc.For_i(0, iters, 1) if iters > 1 else contextlib.nullcontext()
        with loop_cm, ExitStack() as ctx:
            keep = ctx.enter_context(tc.tile_pool(name="keep", bufs=1))

            ident = keep.tile([128, 128], bf16, tag="ident")
            make_identity(nc, ident)

            # persistent activation/weight tiles
            qkT = [keep.tile([128, KVP], bf16, tag=f"qkT{m}", name=f"qkT{m}") for m in range(8)]
            vt = [keep.tile([128, H, D + 1], bf16, tag=f"vt{t}", name=f"vt{t}") for t in range(NT)]
            mC = keep.tile([128, NBLK, 256], bf16, tag="mC")
            ctxT = [keep.tile([128, NQ], f32r, tag=f"ctxT{c}", name=f"ctxT{c}") for c in range(4)]
            woT = [keep.tile([128, 512], f32r, tag=f"woT{k}", name=f"woT{k}") for k in range(4)]

            # ---------------- Phase A: projections ----------------
            with tc.tile_pool(name="aload", bufs=1) as aload, \
                 tc.tile_pool(name="apsum", bufs=4, space="PSUM") as apsum:
                wqk = [aload.tile([128, 1536], f32r, tag=f"wqk{k}", name=f"wqk{k}") for k in range(4)]
                hid = [aload.tile([128, KVP], f32r, tag=f"hid{k}", name=f"hid{k}") for k in range(4)]
                qkb = aload.tile([128, 8], f32, tag="qkb")
                vbb = aload.tile([128, 512], f32, tag="vbb")
                for k in range(4):
                    nc.sync.dma_start(wqk[k][:], wqkT_d[k])
                    nc.sync.dma_start(hid[k][:], hidT_d[k])
                nc.sync.dma_start(qkb[:], qkbT_d)
                nc.sync.dma_start(vbb[:], vbb_d)
                nc.sync.dma_start(mC[:], mC_d)
                for k in range(4):
                    nc.sync.dma_start(woT[k][:], woT_d[k])

                # A1: Q^T, K^T feature-major (feat chunk m on partitions)
                for m in (0, 4, 1, 5, 2, 6, 3, 7):
                    for n0, nsz in A1_TILES:
                        ps = apsum.tile([128, 512], f32, tag="aps")
                        for k in range(4):
                            nc.tensor.matmul(
                                ps[:, :nsz],
                                wqk[k][:, m * 128:(m + 1) * 128],
                                hid[k][:, n0:n0 + nsz],
                                start=(k == 0), stop=(k == 3),
                            )
                        nc.scalar.activation(
                            qkT[m][:, n0:n0 + nsz], ps[:, :nsz],
                            AF.Identity, bias=qkb[:, m:m + 1],
                        )

                # A2: V token-major (+ ones column for the softmax denominator)
                for t in range(NT):
                    ps = apsum.tile([128, 512], f32, tag="aps")
                    for k in range(4):
                        nc.tensor.matmul(
                            ps,
                            hid[k][:, t * 128:(t + 1) * 128],
                            wqk[k][:, 1024:1536],
                            start=(k == 0), stop=(k == 3),
                        )
                    nc.vector.tensor_add(
                        vt[t][:, :, 0:D],
                        ps[:].rearrange("p (h d) -> p h d", d=D),
                        vbb[:].rearrange("p (h d) -> p h d", d=D),
                    )
                    nc.vector.memset(vt[t][:, :, D:D + 1], 1.0)

            # ---------------- Phase B: windowed attention ----------------
            # Scores are computed TRANSPOSED (S^T = K_chunk @ Q^T), so P^T comes
            # straight out of exp with no PE transposes; the band/validity mask
            # is applied multiplicatively on GpSimd (idle otherwise); the
            # softmax denominator falls out of the ones-column in V.
            with tc.tile_pool(name="bwork", bufs=3) as bwork, \
                 tc.tile_pool(name="bpsum", bufs=2, space="PSUM") as bp, \
                 tc.tile_pool(name="scpsum", bufs=2, space="PSUM") as scp:
                for h in range(H):
                    qtile = qkT[h // 2]
                    ktile = qkT[4 + h // 2]
                    prow = (h % 2) * 64
                    for p in range(NPASS):
                        sc = scp.tile([128, PB, 256], f32, tag="sc")
                        for j in range(PB):
                            gb = p * PB + j
                            q0 = HALO + gb * 128
                            nc.tensor.matmul(
                                sc[:, j, 0:128],
                                ktile[prow:prow + 64, gb * 128: gb * 128 + 128],
                                qtile[prow:prow + 64, q0: q0 + 128],
                                start=True, stop=True,
                            )
                            nc.tensor.matmul(
                                sc[:, j, 128:256],
                                ktile[prow:prow + 64, gb * 128 + 128: gb * 128 + 256],
                                qtile[prow:prow + 64, q0: q0 + 128],
                                start=True, stop=True,
                            )
                        P = bwork.tile([128, PB, 256], bf16, tag="P")
                        nc.scalar.activation(P[:], sc[:], AF.Exp)

                        pTs = bwork.tile([128, PB, 256], bf16, tag="pTs")
                        nc.gpsimd.tensor_mul(
                            pTs[:], P[:], mC[:, p * PB:(p + 1) * PB, :])

                        cx = bp.tile([128, PB, 128], f32, tag="cx")
                        for j in range(PB):
                            gb = p * PB + j
                            nc.tensor.matmul(
                                cx[:, j, 0:D + 1], pTs[:, j, 0:128], vt[gb][:, h, :],
                                start=True, stop=False)
                            nc.tensor.matmul(
                                cx[:, j, 0:D + 1], pTs[0:32, j, 128:256], vt[gb + 1][0:32, h, :],
                                start=False, stop=True)

                        z = bwork.tile([128, PB, 1], f32, tag="z")
                        rz = bwork.tile([128, PB, 1], f32, tag="rz")
                        nc.vector.tensor_scalar_max(z[:], cx[:, :, D:D + 1], 1e-30)
                        nc.vector.reciprocal(rz[:], z[:])

                        cxn = bwork.tile([128, PB, D], bf16, tag="cxn")
                        nc.vector.tensor_mul(
                            cxn[:], cx[:, :, 0:D], rz[:].to_broadcast([128, PB, D]))

                        ct = bp.tile([64, PB, 128], bf16, tag="ct")
                        for j in range(PB):
                            nc.tensor.transpose(ct[:, j, :], cxn[:, j, :], ident)
                        nc.vector.tensor_copy(
                            ctxT[h // 2][prow:prow + 64, p * 512:(p + 1) * 512]
                            .rearrange("p (j q) -> p j q", q=128),
                            ct[:])

            # ---------------- Phase C: output projection ----------------
            with tc.tile_pool(name="cwork", bufs=4) as cwork, \
                 tc.tile_pool(name="cpsum", bufs=4, space="PSUM") as cpsum:
                for hc in range(4):
                    for n in range(4):
                        ps = cpsum.tile([128, 512], f32, tag="ops")
                        for k in range(4):
                            nc.tensor.matmul(
                                ps,
                                woT[k][:, hc * 128:(hc + 1) * 128],
                                ctxT[k][:, n * 512:(n + 1) * 512],
                                start=(k == 0), stop=(k == 3),
                            )
                        ot = cwork.tile([128, 512], f32, tag="ot")
                        if (hc * 4 + n) % 2 == 0:
                            nc.scalar.copy(ot[:], ps[:])
                        else:
                            nc.vector.tensor_copy(ot[:], ps[:])
                        nc.sync.dma_start(outT_d[hc, :, n * 512:(n + 1) * 512], ot[:])

    nc.compile()
    return nc


def _get_compiled():
    global _compiled
    if _compiled is None:
        _compiled = _build()
    return _compiled


_bench_cache = {}


def _get_bench(iters):
    if iters not in _bench_cache:
        _bench_cache[iters] = _build(iters)
    return _bench_cache[iters]


def _prep_core_inputs(hidden, valid_mask, wqkT4, woT4, qkbT, vbb):
    """Build the 8 per-core input dicts."""
    import ml_dtypes
    ml_bf16 = np.dtype(ml_dtypes.bfloat16)
    f32 = np.float32
    in_maps = []
    for c in range(NCORE):
        b, s = c // 2, c % 2
        start = s * NQ
        chunk = np.zeros((KVP, HID), dtype=f32)
        if s == 0:
            chunk[HALO:HALO + NQ] = hidden[b, 0:NQ]
        else:
            chunk[0:HALO + NQ] = hidden[b, start - HALO:start + NQ]
        hidT = np.ascontiguousarray(chunk.T).reshape(4, 128, KVP)

        kvr = np.arange(256)[:, None, None]   # kv offset within the 2-chunk span
        gb = np.arange(NBLK)[None, :, None]
        q = np.arange(128)[None, None, :]
        band = (kvr >= q) & (kvr <= q + HALO)
        pos = start + gb * 128 + kvr - HALO + 0 * q
        pos = np.broadcast_to(pos, (256, NBLK, 128))
        ok = (pos >= 0) & (pos < L)
        vm = valid_mask[b][np.clip(pos, 0, L - 1)]
        m = (band & ok & vm)
        # layout: [kv_rel 0:128 -> chunk1 rows, cols 0:128 = q] then chunk2
        mC = np.zeros((128, NBLK, 256), dtype=ml_bf16)
        mC[:, :, 0:128] = m[0:128].astype(ml_bf16)
        mC[:, :, 128:256] = m[128:256].astype(ml_bf16)
        in_maps.append({
            "hidT": hidT,
            "wqkT": wqkT4,
            "woT": woT4,
            "qkbT": qkbT,
            "vbb": vbb,
            "maskC": mC,
        })
    return in_maps


def kernel(hidden, valid_mask, qkv_w, qkv_b, out_w, out_b):
    from concourse.bass_utils import run_bass_kernel_spmd

    f32 = np.float32
    hidden = np.asarray(hidden, dtype=f32)
    valid_mask = np.asarray(valid_mask).astype(bool)
    qkv_w = np.asarray(qkv_w, dtype=f32)
    qkv_b = np.asarray(qkv_b, dtype=f32)
    out_w = np.asarray(out_w, dtype=f32)
    out_b = np.asarray(out_b, dtype=f32)

    # fold the attention scale into Wq/Wk/bq/bk
    qkv_w_s = qkv_w.copy()
    qkv_b_s = qkv_b.copy()
    qkv_w_s[0:2 * ATTN] *= SQS
    qkv_b_s[0:2 * ATTN] *= SQS

    wqkT4 = np.ascontiguousarray(qkv_w_s.T).reshape(4, 128, 3 * ATTN)
    woT4 = np.ascontiguousarray(out_w.T).reshape(4, 128, HID)
    qkbT = np.ascontiguousarray(qkv_b_s[0:2 * ATTN].reshape(8, 128).T)
    vbb = np.ascontiguousarray(
        np.broadcast_to(qkv_b_s[2 * ATTN:3 * ATTN], (128, ATTN)))

    nc = _get_compiled()
    in_maps = _prep_core_inputs(hidden, valid_mask, wqkT4, woT4, qkbT, vbb)
    results = run_bass_kernel_spmd(nc, in_maps, list(range(NCORE))).results

    out = np.empty((B, L, HID), dtype=f32)
    for c in range(NCORE):
        b, s = c // 2, c % 2
        outT = results[c]["outT"].reshape(HID, NQ)
        out[b, s * NQ:(s + 1) * NQ] = outT.T
    out += out_b
    return out


# revision 18
# speedup vs baseline: 2.4351x; 2.4351x over previous
"""Local (sliding-window, causal) self-attention for Trainium2, SPMD over 8 NeuronCores.

Problem: B=4, L=4096, HIDDEN=512, 8 heads x 64 dim, causal window 32.

Sharding: data-parallel over (batch, sequence-half) -> 8 shards. Local attention
only needs a (window-1)=31-token left halo of K/V context, so each core is fully
independent (no collectives).

Device-side layout strategy (everything contracts over the SBUF partition dim):
  - Host pre-transposes activations/weights, so the QKV projection produces
    Q^T/K^T feature-major directly and V token-major, with no device transposes.
  - fp32r matmuls (full PE rate at N>=256, ~tf32 precision) for the three
    projections; bf16 for the windowed attention matmuls.
  - Scores for a 128-query block against its 160-wide key span; softmax without
    max-subtraction (scores are O(1) by construction); exp -> P; P^T via PE
    transpose; band+validity mask applied multiplicatively during the P^T
    eviction; softmax denominator = ones-column augmented into V so it falls out
    of the PV matmul per-partition; ctx normalized token-major, transposed on PE
    to feature-major for the fp32r output projection; result DMA'd out
    feature-major and un-transposed on the host.
"""

import numpy as np

B, L, HID = 4, 4096, 512
H, D, W, ATTN = 8, 64, 32, 512
NCORE = 8
NQ = L // 2                 # queries per core
HALO = W - 1                # 31
KV = HALO + NQ + 1          # 2080 kv positions incl. 1 right pad
KVP = 2176                  # hidT width, padded to 17*128
NT = KVP // 128             # 17 V token chunks
NBLK = NQ // 128            # 16 query blocks
SPAN = 160                  # key span per query block
PB = 4                      # blocks per attention pass
NPASS = NBLK // PB          # 4 passes per head
SQS = float(D) ** -0.25     # sqrt of attention scale, folded into Wq,Wk,bq,bk

A1_TILES = [(0, 512), (512, 512), (1024, 512), (1536, 512), (2048, 128)]

_compiled = None


def _build(iters=1):
    from contextlib import ExitStack

    import concourse.bass as bass
    import concourse.mybir as mybir
    import concourse.tile as tile
    from concourse import bacc
    from concourse.masks import make_identity

    f32 = mybir.dt.float32
    f32r = mybir.dt.float32r
    bf16 = mybir.dt.bfloat16
    AF = mybir.ActivationFunctionType

    nc = bacc.Bacc("TRN2", target_bir_lowering=False, debug=False,
                   num_devices=NCORE)

    hidT_d = nc.dram_tensor("hidT", [4, 128, KVP], bf16, kind="ExternalInput").ap()
    wqkT_d = nc.dram_tensor("wqkT", [4, 128, 1536], bf16, kind="ExternalInput").ap()
    woT_d = nc.dram_tensor("woT", [4, 128, 512], f32r, kind="ExternalInput").ap()
    qkbT_d = nc.dram_tensor("qkbT", [128, 8], f32, kind="ExternalInput").ap()
    vbb_d = nc.dram_tensor("vbb", [128, 512], f32, kind="ExternalInput").ap()
    mC_d = nc.dram_tensor("maskC", [128, NBLK, 256], bf16, kind="ExternalInput").ap()
    outT_d = nc.dram_tensor("outT", [4, 128, NQ], f32, kind="ExternalOutput").ap()

    def r(ap):
        return ap.bitcast(f32r)

    with tile.TileContext(nc) as tc:
        import contextlib
        loop_cm = tc.For_i(0, iters, 1) if iters > 1 else contextlib.nullcontext()
        with loop_cm, ExitStack() as ctx:
            keep = ctx.enter_context(tc.tile_pool(name="keep", bufs=1))

            ident = keep.tile([128, 128], bf16, tag="ident")
            make_identity(nc, ident)

            # persistent activation/weight tiles
            qkT = [keep.tile([128, KVP], bf16, tag=f"qkT{m}", name=f"qkT{m}") for m in range(8)]
            vt = [keep.tile([128, H, D + 1], bf16, tag=f"vt{t}", name=f"vt{t}") for t in range(NT)]
            mC = keep.tile([128, NBLK, 256], bf16, tag="mC")
            ctxT = [keep.tile([128, NQ], f32r, tag=f"ctxT{c}", name=f"ctxT{c}") for c in range(4)]
            woT = [keep.tile([128, 512], f32r, tag=f"woT{k}", name=f"woT{k}") for k in range(4)]

            attn_tail_ref = [None]
            # ---------- Phases A+B fused: projections + attention ----------
            # V is projected first, then A1 feature-chunk pairs are interleaved
            # with the attention heads they feed, so attention's ACT/DVE/Pool
            # work overlaps A1's PE-bound matmuls.
            with tc.tile_pool(name="aload", bufs=1) as aload, \
                 tc.tile_pool(name="apsum", bufs=2, space="PSUM") as apsum, \
                 tc.tile_pool(name="bwork", bufs=3) as bwork, \
                 tc.tile_pool(name="bpsum", bufs=1, space="PSUM") as bp, \
                 tc.tile_pool(name="scpsum", bufs=2, space="PSUM") as scp:
                wqk = [aload.tile([128, 1536], bf16, tag=f"wqk{k}", name=f"wqk{k}") for k in range(4)]
                hid = [aload.tile([128, KVP], bf16, tag=f"hid{k}", name=f"hid{k}") for k in range(4)]
                qkb = aload.tile([128, 8], f32, tag="qkb")
                vbb = aload.tile([128, 512], f32, tag="vbb")
                for k in range(4):
                    nc.sync.dma_start(wqk[k][:], wqkT_d[k])
                    nc.sync.dma_start(hid[k][:, 0:1024], hidT_d[k][:, 0:1024])
                for k in range(4):
                    nc.sync.dma_start(hid[k][:, 1024:KVP], hidT_d[k][:, 1024:KVP])
                nc.sync.dma_start(qkb[:], qkbT_d)
                nc.sync.dma_start(vbb[:], vbb_d)

                def a2_chunk(t):
                    ps = apsum.tile([128, 512], f32, tag="aps", name=f"psA2_{t}")
                    for k in range(4):
                        nc.tensor.matmul(
                            ps,
                            hid[k][:, t * 128:(t + 1) * 128],
                            wqk[k][:, 1024:1536],
                            start=(k == 0), stop=(k == 3),
                        )
                    nc.vector.tensor_add(
                        vt[t][:, :, 0:D],
                        ps[:].rearrange("p (h d) -> p h d", d=D),
                        vbb[:].rearrange("p (h d) -> p h d", d=D),
                    )
                    nc.vector.memset(vt[t][:, :, D:D + 1], 1.0)

                def a1_chunk(m):
                    for ni, (n0, nsz) in enumerate(A1_TILES):
                        ps = apsum.tile([128, 512], f32, tag="aps", name=f"psA1_{m}_{n0}")
                        for k in range(4):
                            nc.tensor.matmul(
                                ps[:, :nsz],
                                wqk[k][:, m * 128:(m + 1) * 128],
                                hid[k][:, n0:n0 + nsz],
                                start=(k == 0), stop=(k == 3),
                            )
                        if ni % 2 == 0:
                            nc.scalar.activation(
                                qkT[m][:, n0:n0 + nsz], ps[:, :nsz],
                                AF.Identity, bias=qkb[:, m:m + 1],
                            )
                        else:
                            nc.vector.tensor_scalar_add(
                                qkT[m][:, n0:n0 + nsz], ps[:, :nsz],
                                qkb[:, m:m + 1])

                def attn_head(h, bwork, bp, scp):
                    qtile = qkT[h // 2]
                    ktile = qkT[4 + h // 2]
                    prow = (h % 2) * 64
                    for p in range(NPASS):
                        sc = scp.tile([128, PB, 256], f32, tag="sc",
                                      name=f"sc_{h}_{p}")
                        for j in range(PB):
                            gb = p * PB + j
                            q0 = HALO + gb * 128
                            nc.tensor.matmul(
                                sc[:, j, 0:128],
                                ktile[prow:prow + 64, gb * 128: gb * 128 + 128],
                                qtile[prow:prow + 64, q0: q0 + 128],
                                start=True, stop=True,
                            )
                            nc.tensor.matmul(
                                sc[:, j, 128:256],
                                ktile[prow:prow + 64, gb * 128 + 128: gb * 128 + 256],
                                qtile[prow:prow + 64, q0: q0 + 128],
                                start=True, stop=True,
                            )
                        P = bwork.tile([128, PB, 256], bf16, tag="P",
                                       name=f"P_{h}_{p}")
                        nc.scalar.activation(P[:], sc[:], AF.Exp)

                        # band/validity mask, multiplicative: chunk1 on GpSimd,
                        # chunk2 (32 rows) on DVE
                        pTs = bwork.tile([128, PB, 256], bf16, tag="pTs",
                                         name=f"pTs_{h}_{p}")
                        nc.gpsimd.tensor_mul(
                            pTs[:, :, 0:128], P[:, :, 0:128],
                            mC[:, p * PB:(p + 1) * PB, 0:128])
                        nc.gpsimd.tensor_mul(
                            pTs[0:32, :, 128:256], P[0:32, :, 128:256],
                            mC[0:32, p * PB:(p + 1) * PB, 128:256])

                        cx = bp.tile([128, PB, 128], f32, tag="cx",
                                     name=f"cx_{h}_{p}")
                        for j in range(PB):
                            gb = p * PB + j
                            nc.tensor.matmul(
                                cx[:, j, 0:D + 1], pTs[:, j, 0:128], vt[gb][:, h, :],
                                start=True, stop=False)
                            nc.tensor.matmul(
                                cx[:, j, 0:D + 1], pTs[0:32, j, 128:256], vt[gb + 1][0:32, h, :],
                                start=False, stop=True)

                        rz = bwork.tile([128, PB, 1], f32, tag="rz",
                                        name=f"rz_{h}_{p}")
                        nc.vector.reciprocal(rz[:], cx[:, :, D:D + 1])

                        cxn = bwork.tile([128, PB, D], bf16, tag="cxn",
                                         name=f"cxn_{h}_{p}")
                        nc.vector.tensor_mul(
                            cxn[:], cx[:, :, 0:D], rz[:].to_broadcast([128, PB, D]))

                        ct = bp.tile([64, PB, 128], bf16, tag="ct",
                                     name=f"ct_{h}_{p}")
                        for j in range(PB):
                            nc.tensor.transpose(ct[:, j, :], cxn[:, j, :], ident)
                        nc.vector.tensor_copy(
                            ctxT[h // 2][prow:prow + 64, p * 512:(p + 1) * 512]
                            .rearrange("p (j q) -> p j q", q=128),
                            ct[:])

                def attn_tail(bw, bpp, scpp):
                    attn_head(7, bw, bpp, scpp)

                attn_tail_ref[0] = attn_tail

                for t in range(NT):
                    a2_chunk(t)
                for hi in range(4):
                    a1_chunk(hi)
                    if hi == 0:
                        nc.sync.dma_start(mC[:], mC_d)
                        for k in range(4):
                            nc.sync.dma_start(woT[k][:], woT_d[k])
                    a1_chunk(4 + hi)
                    attn_head(2 * hi, bwork, bp, scp)
                    if hi < 3:
                        attn_head(2 * hi + 1, bwork, bp, scp)

            # attention tail overlapped with the output projection: the
            # out-proj k<3 contraction chunks only need earlier heads' ctxT.
            with tc.tile_pool(name="bwork2", bufs=3) as bwork2, \
                 tc.tile_pool(name="bpsum2", bufs=1, space="PSUM") as bp2, \
                 tc.tile_pool(name="scpsum2", bufs=1, space="PSUM") as scp2, \
                 tc.tile_pool(name="cwork", bufs=4) as cwork, \
                 tc.tile_pool(name="cpsum", bufs=4, space="PSUM") as cpsum:
                attn_tail_ref[0](bwork2, bp2, scp2)
                for hc in range(4):
                    for n in range(4):
                        ps = cpsum.tile([128, 512], f32, tag="ops")
                        for k in range(4):
                            nc.tensor.matmul(
                                ps,
                                woT[k][:, hc * 128:(hc + 1) * 128],
                                ctxT[k][:, n * 512:(n + 1) * 512],
                                start=(k == 0), stop=(k == 3),
                            )
                        ot = cwork.tile([128, 512], f32, tag="ot")
                        if (hc * 4 + n) % 2 == 0:
                            nc.scalar.copy(ot[:], ps[:])
                        else:
                            nc.vector.tensor_copy(ot[:], ps[:])
                        nc.sync.dma_start(outT_d[hc, :, n * 512:(n + 1) * 512], ot[:])

    nc.compile()
    return nc


def _get_compiled():
    global _compiled
    if _compiled is None:
        _compiled = _build()
    return _compiled


_bench_cache = {}


def _get_bench(iters):
    if iters not in _bench_cache:
        _bench_cache[iters] = _build(iters)
    return _bench_cache[iters]


def _prep_core_inputs(hidden, valid_mask, wqkT4, woT4, qkbT, vbb):
    """Build the 8 per-core input dicts."""
    import ml_dtypes
    ml_bf16 = np.dtype(ml_dtypes.bfloat16)
    f32 = np.float32
    in_maps = []
    for c in range(NCORE):
        b, s = c // 2, c % 2
        start = s * NQ
        chunk = np.zeros((KVP, HID), dtype=f32)
        if s == 0:
            chunk[HALO:HALO + NQ] = hidden[b, 0:NQ]
        else:
            chunk[0:HALO + NQ] = hidden[b, start - HALO:start + NQ]
        hidT = np.ascontiguousarray(chunk.T).astype(ml_bf16).reshape(4, 128, KVP)

        kvr = np.arange(256)[:, None, None]   # kv offset within the 2-chunk span
        gb = np.arange(NBLK)[None, :, None]
        q = np.arange(128)[None, None, :]
        band = (kvr >= q) & (kvr <= q + HALO)
        pos = start + gb * 128 + kvr - HALO + 0 * q
        pos = np.broadcast_to(pos, (256, NBLK, 128))
        ok = (pos >= 0) & (pos < L)
        vm = valid_mask[b][np.clip(pos, 0, L - 1)]
        m = (band & ok & vm)
        # layout: [kv_rel 0:128 -> chunk1 rows, cols 0:128 = q] then chunk2
        mC = np.zeros((128, NBLK, 256), dtype=ml_bf16)
        mC[:, :, 0:128] = m[0:128].astype(ml_bf16)
        mC[:, :, 128:256] = m[128:256].astype(ml_bf16)
        in_maps.append({
            "hidT": hidT,
            "wqkT": wqkT4,
            "woT": woT4,
            "qkbT": qkbT,
            "vbb": vbb,
            "maskC": mC,
        })
    return in_maps


def kernel(hidden, valid_mask, qkv_w, qkv_b, out_w, out_b):
    from concourse.bass_utils import run_bass_kernel_spmd

    f32 = np.float32
    hidden = np.asarray(hidden, dtype=f32)
    valid_mask = np.asarray(valid_mask).astype(bool)
    qkv_w = np.asarray(qkv_w, dtype=f32)
    qkv_b = np.asarray(qkv_b, dtype=f32)
    out_w = np.asarray(out_w, dtype=f32)
    out_b = np.asarray(out_b, dtype=f32)

    # fold the attention scale into Wq/Wk/bq/bk
    qkv_w_s = qkv_w.copy()
    qkv_b_s = qkv_b.copy()
    qkv_w_s[0:2 * ATTN] *= SQS
    qkv_b_s[0:2 * ATTN] *= SQS

    import ml_dtypes
    wqkT4 = np.ascontiguousarray(qkv_w_s.T).astype(ml_dtypes.bfloat16).reshape(4, 128, 3 * ATTN)
    woT4 = np.ascontiguousarray(out_w.T).reshape(4, 128, HID)
    qkbT = np.ascontiguousarray(qkv_b_s[0:2 * ATTN].reshape(8, 128).T)
    vbb = np.ascontiguousarray(
        np.broadcast_to(qkv_b_s[2 * ATTN:3 * ATTN], (128, ATTN)))

    nc = _get_compiled()
    in_maps = _prep_core_inputs(hidden, valid_mask, wqkT4, woT4, qkbT, vbb)
    results = run_bass_kernel_spmd(nc, in_maps, list(range(NCORE))).results

    out = np.empty((B, L, HID), dtype=f32)
    for c in range(NCORE):
        b, s = c // 2, c % 2
        outT = results[c]["outT"].reshape(HID, NQ)
        out[b, s * NQ:(s + 1) * NQ] = outT.T
    out += out_b
    return out
